# revision 30
# baseline (speedup 1.0000x reference)
"""Trainium2 Bass kernel v6 for nn_CategoricalDecoder (topk_masking).

Single-pass bin-sharded design. The one-hot pick matrices are folded into
the net weights on the host (V = W2 @ oh), so head/tail pick scores are
matmuls straight off h — no logit copies. The logits path survives only
for the per-feature logsumexp. Per batch row the local top-8 (tail, head)
pairs ride a 16KB AllToAll; the merge is a threshold-masked logsumexp.
"""

import numpy as np
from contextlib import ExitStack

import bass_rust as _br
import concourse.bass as bass
import concourse.bacc as bacc
import concourse.tile as tile
from concourse import mybir
from concourse.bass_utils import run_bass_kernel_spmd
from concourse.hw_specs import get_activation_tables

F32 = mybir.dt.float32
F32R = mybir.dt.float32r
U16 = mybir.dt.uint16
I16 = mybir.dt.int16
AF = mybir.ActivationFunctionType
ALU = mybir.AluOpType
AX = mybir.AxisListType

B, N, Lz, H, D, C = 256, 8192, 64, 256, 32, 16
DC = D * C
P = 8
NL = N // P
BL = B // P
K = 16
NEG = -1.0e30

# pk64 column offsets (partition dim 65: row 64 carries b1 under W1H and
# ones under ZTSH, folding the h bias into the matmul contraction)
O_W1H, O_W1L, O_ZTSH, O_ZTSL = 0, 256, 512, 1536
PK64_COLS = 2560
# pk128 column offsets, ordered by first use so split DMAs land just-in-time
O_W2R, O_B2C, O_B2G, O_GSEL, O_GS3 = 0, 1024, 1028, 1030, 1158
O_W2TH, O_W2TL, O_VTH, O_VTL = 1222, 1350, 1478, 1990
O_VH, O_COEFH, O_COEFT, O_MASK, O_CBT = 2502, 3014, 3142, 3270, 3398
PK128_COLS = 3399


class _Bacc(bacc.Bacc):
    """Bacc that pins every activation to the one table holding
    {Relu, Exp, Ln, Copy}, avoiding per-switch ACT_TABLE_LOADs."""

    def insert_act_table_loads(self):
        has_act = any(isinstance(i, mybir.InstActivation)
                      for b in self.main_func.blocks for i in b.instructions)
        if not has_act:
            return
        tables = []
        for name, funcs in get_activation_tables(self.m.arch).items():
            keep = funcs if name == "natural_log_exp_and_others" else set()
            tables.append((name, keep))
        _br.insert_act_table_loads(self, tables)


def _build_nc():
    nc = _Bacc("TRN2", target_bir_lowering=False, num_devices=P)

    dp = nc.declare_dram_parameter
    pk64 = dp("pk64", [Lz + 1, PK64_COLS], F32R, isOutput=False)
    pk128 = dp("pk128", [128, PK128_COLS], F32R, isOutput=False)
    outp = dp("out", [BL], F32, isOutput=True)

    with tile.TileContext(nc) as tc, ExitStack() as ctx:
        const = ctx.enter_context(tc.tile_pool(name="const", bufs=1))
        dram = ctx.enter_context(tc.tile_pool(name="dram", bufs=1, space="DRAM"))

        k64 = const.tile([Lz + 1, PK64_COLS], F32R, name="k64")
        nc.sync.dma_start(k64[:, 0:O_ZTSL], pk64[:, 0:O_ZTSL])
        nc.sync.dma_start(k64[:, O_ZTSL:], pk64[:, O_ZTSL:])
        k128 = const.tile([128, PK128_COLS], F32R, name="k128")
        nc.sync.dma_start(k128[:, 0:O_W2TH], pk128[:, 0:O_W2TH])
        nc.sync.dma_start(k128[:, O_W2TH:O_VH], pk128[:, O_W2TH:O_VH])
        nc.sync.dma_start(k128[:, O_VH:], pk128[:, O_VH:])

        def c64(off, w, p=Lz, dt=None):
            ap = k64[0:p, off:off + w]
            return ap.bitcast(dt) if dt else ap

        def c128(off, w, p=128, dt=None):
            ap = k128[0:p, off:off + w]
            return ap.bitcast(dt) if dt else ap

        xin = dram.tile([B, 16], F32)
        xout = dram.tile([B, 16], F32)

        # warmup: junk matmuls bridge the DMA wait so HAM reaches 2.4 GHz
        # before real work; dummy activation pulls in the ACT table load;
        # dummy ap_gather pre-loads the gpsimd gather library.
        with ExitStack() as ctxW:
            wsb = ctxW.enter_context(tc.tile_pool(name="wsb", bufs=1))
            wps = ctxW.enter_context(tc.tile_pool(name="wps", bufs=2, space="PSUM"))
            wj = wsb.tile([128, 512], F32, name="wj")
            nc.vector.memset(wj[:], 0)
            wa = wsb.tile([1, 2], F32, name="wa")
            nc.scalar.activation(wa[:], wj[0:1, 0:2], AF.Exp)
            for g in range(2):
                wp = wps.tile([128, 512], F32, tag="wp")
                for i in range(5):
                    nc.tensor.matmul(wp[:], wj[:, 0:128].bitcast(F32R),
                                     wj[:].bitcast(F32R),
                                     start=(i == 0), stop=(i == 4))
            zidx = wsb.tile([16, 4], I16, name="zidx")
            nc.vector.memset(zidx[:], 0)
            jtab = wsb.tile([16, 64], F32, name="jtab")
            nc.vector.memset(jtab[:], 0)
            junkg = wsb.tile([16, 64], F32, name="junkg")
            nc.gpsimd.ap_gather(junkg[:], jtab[:], zidx[:],
                                channels=16, num_elems=64, d=1, num_idxs=64)

        act = ctx.enter_context(tc.tile_pool(name="act", bufs=1))
        scr = ctx.enter_context(tc.tile_pool(name="scr", bufs=3))

        hh = [act.tile([128, NL], F32R, name=f"hh{m}") for m in range(2)]
        hl = [act.tile([128, NL], F32R, name=f"hl{m}") for m in range(2)]
        lgh = act.tile([32, NL], F32R, name="lgh")
        lgf = act.tile([32, NL], F32, name="lgf")
        lgl = act.tile([32, NL], F32R, name="lgl")
        SL = [slice(0, 512), slice(512, 1024)]

        # ---------------- h = relu(W1.T @ zT + b1) ----------------
        with ExitStack() as ctxH:
            php = ctxH.enter_context(tc.tile_pool(name="php", bufs=4, space="PSUM"))
            ph = [[php.tile([128, 512], F32, tag="ph", name=f"ph{m}{f}")
                   for f in range(2)] for m in range(2)]
            # contraction is 65 rows: row 64 of W1H holds b1, row 64 of ZTSH
            # holds ones (W1L/ZTSL row 64 are zero), so b1 folds in once.
            for trm in range(3):
                for m in range(2):
                    w1h = c64(O_W1H + m * 128, 128, p=Lz + 1)
                    w1l = c64(O_W1L + m * 128, 128, p=Lz + 1)
                    w, zc = [(w1h, O_ZTSH), (w1l, O_ZTSH), (w1h, O_ZTSL)][trm]
                    for f in range(2):
                        nc.tensor.matmul(ph[m][f][:], w,
                                         c64(zc + f * 512, 512, p=Lz + 1),
                                         start=(trm == 0), stop=(trm == 2))
            for m in range(2):
                for f in range(2):
                    nc.scalar.activation(hh[m][:, SL[f]], ph[m][f][:], AF.Relu)
                    nc.vector.scalar_tensor_tensor(
                        hl[m][:, SL[f]], ph[m][f][:], 0.0,
                        hh[m][:, SL[f]].bitcast(F32),
                        op0=ALU.max, op1=ALU.subtract)

        # tail/head pick scores accumulate here while the lse path runs
        pso = ctx.enter_context(tc.tile_pool(name="pso", bufs=2, space="PSUM"))
        pst = [pso.tile([128, NL], F32, tag="pst", name=f"pst{bt}")
               for bt in range(2)]

        # ---------------- logits -> exp -> per-feature lse ----------------
        with ExitStack() as ctxL:
            plp = ctxL.enter_context(tc.tile_pool(name="plp", bufs=2, space="PSUM"))
            psep = ctxL.enter_context(tc.tile_pool(name="psep", bufs=2, space="PSUM"))
            pse = [psep.tile([32, 512], F32, tag="pse", name=f"pse{f}")
                   for f in range(2)]
            for t in range(3):
                pl = [plp.tile([128, 512], F32, tag="pl", name=f"pl{t}{f}")
                      for f in range(2)]
                for kk in range(2):
                    w2 = c128(O_W2R + kk * 512 + t * 128, 128)
                    for f in range(2):
                        nc.tensor.matmul(pl[f][:], w2, hh[kk][:, SL[f]],
                                         start=(kk == 0), stop=(kk == 1))
                for f in range(2):
                    e = scr.tile([128, 512], F32R, tag="e")
                    nc.scalar.activation(e[:], pl[f][:], AF.Exp,
                                         bias=c128(O_B2C + t, 1, dt=F32))
                    nc.tensor.matmul(pse[f][:], c128(O_GSEL + t * 32, 32), e[:],
                                     start=(t == 0), stop=False)
            # t3 head half (dc 384..447), 1-term
            pl3h = [plp.tile([Lz, 512], F32, tag="pl", name=f"pl3h{f}")
                    for f in range(2)]
            for kk in range(2):
                w2 = c128(O_W2R + kk * 512 + 384, 64)
                for f in range(2):
                    nc.tensor.matmul(pl3h[f][:], w2, hh[kk][:, SL[f]],
                                     start=(kk == 0), stop=(kk == 1))
            for f in range(2):
                e3h = scr.tile([Lz, 512], F32R, tag="e")
                nc.scalar.activation(e3h[:], pl3h[f][:], AF.Exp,
                                     bias=c128(O_B2G, 1, p=Lz, dt=F32))
                nc.tensor.matmul(pse[f][:], c128(O_GS3, 32, p=Lz), e3h[:],
                                 start=False, stop=False)
            # t3 tail half (dc 448..511), 3-term for selection precision
            pl3t = [plp.tile([Lz, 512], F32, tag="pl", name=f"pl3t{f}")
                    for f in range(2)]
            for kk in range(2):
                w2h = c128(O_W2TH + kk * 64, 64)
                w2l = c128(O_W2TL + kk * 64, 64)
                for trm in range(3):
                    w, hsrc = [(w2h, hh), (w2h, hl), (w2l, hh)][trm]
                    for f in range(2):
                        nc.tensor.matmul(pl3t[f][:], w, hsrc[kk][:, SL[f]],
                                         start=(kk == 0 and trm == 0),
                                         stop=(kk == 1 and trm == 2))
            for f in range(2):
                e3t = scr.tile([Lz, 512], F32R, tag="e")
                nc.scalar.activation(e3t[:], pl3t[f][:], AF.Exp,
                                     bias=c128(O_B2G + 1, 1, p=Lz, dt=F32))
                nc.tensor.matmul(pse[f][:], c128(O_GS3 + 32, 32, p=Lz), e3t[:],
                                 start=False, stop=True)
                nc.scalar.activation(lgh[:, SL[f]], pse[f][:], AF.Ln)
                nc.scalar.activation(lgf[:, SL[f]], pse[f][:], AF.Ln)
                nc.vector.tensor_sub(lgl[:, SL[f]], lgf[:, SL[f]],
                                     lgh[:, SL[f]].bitcast(F32))

            # tail pick scores (V_tail.T @ h, 3-term) — only need hh/hl, so
            # they stream right behind the l matmuls with no extra deps
            for trm in range(6):
                kk = trm // 3
                wt, hsrc = [(O_VTH, hh), (O_VTH, hl), (O_VTL, hh)][trm % 3]
                for bt in range(2):
                    lhsT = c128(wt + kk * 256 + bt * 128, 128)
                    for f in range(2):
                        nc.tensor.matmul(pst[bt][:, SL[f]], lhsT,
                                         hsrc[kk][:, SL[f]],
                                         start=(trm == 0), stop=False)

        # ---------------- scores + local top-8 + payload ----------------
        dsb = [act.tile([128, NL], F32, name=f"dsb{bt}") for bt in range(2)]
        pu = act.tile([128, 16], U16, name="pu")
        xin_sb = [act.tile([128, 16], F32, name=f"xin{bt}") for bt in range(2)]
        with ExitStack() as ctxS:
            pnp = ctxS.enter_context(tc.tile_pool(name="pnp", bufs=2, space="PSUM"))
            pnum = [pnp.tile([128, NL], F32, tag="pnum", name=f"pnum{bt}")
                    for bt in range(2)]
            # both tail folds + selections up front: bt1's select runs on the
            # DVE while bt0's head-score matmuls stream, so gather1 is gated
            # only by dsb1 instead of a late find_index8
            for trm, src in enumerate((lgh, lgl)):
                for bt in range(2):
                    for f in range(2):
                        nc.tensor.matmul(pst[bt][:, SL[f]],
                                         c128(O_COEFT, 128, p=32),
                                         src[:, SL[f]], start=False,
                                         stop=(trm == 1))
            for bt in range(2):
                nc.vector.max(xin_sb[bt][:, 0:8], pst[bt][:])
                nc.vector.max_index(pu[:, bt * 8:bt * 8 + 8],
                                    xin_sb[bt][:, 0:8], pst[bt][:])
            for bt in range(2):
                # head pick scores (V_head.T @ h, 2-term: V exact, h split)
                for trm in range(4):
                    kk = trm // 2
                    hsrc = [hh, hl][trm % 2]
                    lhsT = c128(O_VH + kk * 256 + bt * 128, 128)
                    for f in range(2):
                        nc.tensor.matmul(pnum[bt][:, SL[f]], lhsT,
                                         hsrc[kk][:, SL[f]],
                                         start=(trm == 0), stop=False)
                # head lse fold is 1-term: its f32r rounding (~4e-3 on den)
                # largely cancels between lse(num) and lse(den)
                for f in range(2):
                    nc.tensor.matmul(pnum[bt][:, SL[f]], c128(O_COEFH, 128, p=32),
                                     lgh[:, SL[f]], start=False, stop=True)
                nc.scalar.copy(dsb[bt][:, 0:512], pnum[bt][:, 0:512])
                nc.vector.tensor_copy(dsb[bt][:, 512:1024],
                                      pnum[bt][:, 512:1024])
                G = scr.tile([128, 128], F32, tag="G")
                nc.gpsimd.ap_gather(G[:], dsb[bt][:],
                                    pu[:, bt * 8:bt * 8 + 8].bitcast(I16),
                                    channels=128, num_elems=NL, d=1,
                                    num_idxs=128)
                # head value j of row r sits at G[r, j*16 + r%16]: extract the
                # stride-16 diagonal via mask-multiply + 16-col reduce.
                gm = scr.tile([128, 128], F32, tag="gm")
                nc.vector.tensor_tensor(gm[:], G[:], c128(O_MASK, 128, dt=F32),
                                        op=ALU.mult)
                nc.vector.tensor_reduce(
                    xin_sb[bt][:, 8:16],
                    gm[:].rearrange("p (j q) -> p j q", q=16),
                    axis=AX.X, op=ALU.add)
                nc.sync.dma_start(xin[bt * 128:(bt + 1) * 128, :],
                                  xin_sb[bt][:])

        nc.gpsimd.collective_compute(
            "AllToAll", ALU.bypass, replica_groups=[list(range(P))],
            ins=[xin[:].opt()], outs=[xout[:].opt()],
        )

        # ---------------- merge: threshold top-16, masked logsumexp ----------
        y = act.tile([BL, P, 16], F32, name="y")
        nc.sync.dma_start(y[:], xout[:].rearrange("(s p) f -> p s f", s=P))
        tails = act.tile([BL, P * 8], F32, name="tails")
        nc.vector.tensor_copy(
            tails[:].rearrange("p (a b) -> p a b", a=P), y[:, :, 0:8])
        dens = act.tile([BL, P * 8], F32, name="dens")
        nc.vector.tensor_copy(
            dens[:].rearrange("p (a b) -> p a b", a=P), y[:, :, 8:16])

        wv = act.tile([BL, 16], F32, name="wv")
        nc.vector.max(wv[:, 0:8], tails[:])
        cm = act.tile([BL, P * 8], F32, name="cm")
        nc.vector.match_replace(cm[:], wv[:, 0:8], tails[:], NEG)
        nc.vector.max(wv[:, 8:16], cm[:])

        mask = act.tile([BL, P * 8], F32, name="mask")
        nc.vector.tensor_scalar(mask[:], tails[:], wv[:, 15:16], None,
                                op0=ALU.is_ge)
        num = act.tile([BL, P * 8], F32, name="num")
        nc.vector.tensor_add(num[:], tails[:], dens[:])
        ng = act.tile([BL, 2], F32, name="ng")
        nc.vector.tensor_reduce(ng[:, 0:1], num[:], axis=AX.X, op=ALU.max,
                                negate=True)
        nc.vector.tensor_reduce(ng[:, 1:2], dens[:], axis=AX.X, op=ALU.max,
                                negate=True)
        en = scr.tile([BL, P * 8], F32, tag="ex")
        nc.scalar.activation(en[:], num[:], AF.Exp, bias=ng[:, 0:1])
        ed = scr.tile([BL, P * 8], F32, tag="ex")
        nc.scalar.activation(ed[:], dens[:], AF.Exp, bias=ng[:, 1:2])
        s2 = act.tile([BL, 2], F32, name="s2")
        jm = scr.tile([BL, P * 8], F32, tag="ex")
        nc.vector.scalar_tensor_tensor(jm[:], en[:], 1.0, mask[:],
                                       op0=ALU.mult, op1=ALU.mult,
                                       accum_out=s2[:, 0:1])
        jm2 = scr.tile([BL, P * 8], F32, tag="ex")
        nc.vector.scalar_tensor_tensor(jm2[:], ed[:], 1.0, mask[:],
                                       op0=ALU.mult, op1=ALU.mult,
                                       accum_out=s2[:, 1:2])
        lg = act.tile([BL, 2], F32, name="lg")
        nc.scalar.activation(lg[:], s2[:], AF.Ln)
        t1 = act.tile([BL, 1], F32, name="t1")
        nc.vector.tensor_sub(t1[:], lg[:, 0:1], lg[:, 1:2])
        t2 = act.tile([BL, 1], F32, name="t2")
        nc.vector.tensor_sub(t2[:], ng[:, 1:2], ng[:, 0:1])
        t3 = act.tile([BL, 1], F32, name="t3")
        nc.vector.tensor_add(t3[:], t1[:], t2[:])
        t4 = act.tile([BL, 1], F32, name="t4")
        nc.vector.tensor_add(t4[:], t3[:], c128(O_CBT, 1, p=BL, dt=F32))
        nc.sync.dma_start(outp[:], t4[:, 0])

    nc.compile()
    return nc


def _trunc_split(a):
    a = np.ascontiguousarray(a, np.float32)
    hi = (a.view(np.uint32) & np.uint32(0xFFFFF000)).view(np.float32)
    lo = a - hi
    return hi, lo


def _host_prep(x, z, W1, b1, W2, b2):
    oh = np.zeros((B, DC), np.float32)
    oh[np.arange(B)[:, None], np.arange(D)[None, :] * C + x] = 1.0
    ohT = np.ascontiguousarray(oh.T)
    w2s = np.ascontiguousarray(
        W2.reshape(2, 128, DC).transpose(1, 0, 2).reshape(128, 2 * DC))
    w1h, w1l = _trunc_split(W1)
    cbt = oh[:, 448:512] @ b2[448:512]          # (256,)

    # fold the one-hot picks into the weights: V[h, b] = sum_dc W2[h,dc] oh[dc,b]
    Vh_ = (W2[:, 0:448].astype(np.float64) @ ohT[0:448].astype(np.float64)
           ).astype(np.float32)                 # (H, B) head picks
    Vt_ = (W2[:, 448:512].astype(np.float64) @ ohT[448:512].astype(np.float64)
           ).astype(np.float32)                 # (H, B) tail picks
    vh, vl = _trunc_split(Vh_.reshape(2, 128, B).transpose(1, 0, 2)
                          .reshape(128, 2 * B))
    vth, vtl = _trunc_split(Vt_.reshape(2, 128, B).transpose(1, 0, 2)
                            .reshape(128, 2 * B))

    k64c = np.zeros((Lz + 1, PK64_COLS), np.float32)
    k64c[0:Lz, O_W1H:O_W1H + H] = w1h
    k64c[0:Lz, O_W1L:O_W1L + H] = w1l
    k64c[Lz, O_W1H:O_W1H + H] = b1          # bias row of the h contraction
    k64c[Lz, O_ZTSH:O_ZTSH + NL] = 1.0

    k128c = np.zeros((128, PK128_COLS), np.float32)
    k128c[:, O_VH:O_VH + 2 * B] = vh
    k128c[:, O_VTH:O_VTH + 2 * B] = vth
    k128c[:, O_VTL:O_VTL + 2 * B] = vtl
    k128c[:, O_W2R:O_W2R + 2 * DC] = w2s
    for kk in range(2):
        th, tl = _trunc_split(w2s[:, kk * DC + 448:kk * DC + 512])
        k128c[:, O_W2TH + kk * 64:O_W2TH + (kk + 1) * 64] = th
        k128c[:, O_W2TL + kk * 64:O_W2TL + (kk + 1) * 64] = tl
    p_idx = np.arange(128)
    for t in range(3):
        k128c[p_idx, O_GSEL + t * 32 + t * 8 + p_idx // 16] = 1.0
    p64 = np.arange(Lz)
    k128c[p64, O_GS3 + 24 + p64 // 16] = 1.0
    k128c[p64, O_GS3 + 32 + 28 + p64 // 16] = 1.0
    k128c[0:28, O_COEFH:O_COEFH + 128] = -1.0
    k128c[28:32, O_COEFT:O_COEFT + 128] = -1.0
    # diagonal-extraction mask: M[p, c] = 1 iff c % 16 == p % 16
    k128c[:, O_MASK:O_MASK + 128] = (
        (np.arange(128)[None, :] % 16) == (p_idx % 16)[:, None]
    ).astype(np.float32)
    k128c[:, O_B2C:O_B2C + 4] = b2.reshape(4, 128).T
    k128c[0:Lz, O_B2G:O_B2G + 2] = b2[384:512].reshape(2, Lz).T

    in_maps = []
    for c in range(P):
        kc64 = k64c.copy()
        zsh, zsl = _trunc_split(z[c * NL:(c + 1) * NL, :].T)
        kc64[0:Lz, O_ZTSH:O_ZTSH + NL] = zsh
        kc64[0:Lz, O_ZTSL:O_ZTSL + NL] = zsl
        kc128 = k128c.copy()
        kc128[0:BL, O_CBT] = cbt[c * BL:(c + 1) * BL]
        in_maps.append(dict(pk64=kc64, pk128=kc128))
    return in_maps


_NC_CACHE = {}


def kernel(x, log_w, z, k, W1, b1, W2, b2, _trace=False, _trace_kwargs=None):
    assert int(k) == K
    in_maps = _host_prep(np.asarray(x, np.int32), np.asarray(z, np.float32),
                         np.asarray(W1, np.float32), np.asarray(b1, np.float32),
                         np.asarray(W2, np.float32), np.asarray(b2, np.float32))
    if "nc" not in _NC_CACHE:
        _NC_CACHE["nc"] = _build_nc()
    nc = _NC_CACHE["nc"]
    res = run_bass_kernel_spmd(
        nc, in_maps, list(range(P)), trace=_trace, **(_trace_kwargs or {}))
    if _trace:
        _NC_CACHE["last_result"] = res
    return np.concatenate([np.asarray(res.results[c]["out"], np.float32)
                           for c in range(P)])
